# revision 1
# baseline (speedup 1.0000x reference)
"""CrossSymmetricModal trn2 kernel: 2 cross-attention branches + BN + residual.

Data-parallel over batch (2 samples/core on 8 cores); BatchNorm batch stats
cross-core via a small AllReduce per branch.

Heavy matmuls run in bf16 on the TensorEngine (fp32 PSUM accumulation):
 - convs (K=3, 'same' pad) are tap-shifted matmuls accumulating in PSUM; the
   1->256 convs contract over the 3-row im2col of clinical.
 - attention is transpose-free: scoresT[m,l] = k^T q is produced directly in
   [m (partition), l] orientation, exp on ScalarE (no max subtraction: scores
   are bounded ~|1.6|), the softmax denominator is a ones-vector matmul, and
   ctx[c,l] = vT.T @ expT needs vT which each branch's v-conv emits directly
   in transposed [m, c] layout.
The residual image, all reductions and the BN math stay in fp32.
"""
import os
import sys

sys.path.insert(0, '/opt/trn_rl_repo')

import ml_dtypes
import numpy as np

from concourse import bacc, mybir, tile
from concourse.bass_utils import run_bass_kernel_spmd

S = 2            # samples per core
NCORES = 8
C = 256
CT = 2           # 128-partition channel tiles
L = 1024
LS = 2           # 512-wide l slices
EPS = 1e-5
SCALE = 1.0 / 16.0   # 1/sqrt(C)
NSTAT = 16 * L       # batchnorm reduction size (full batch x length)

F32 = mybir.dt.float32
F32R = mybir.dt.float32r
BF16 = mybir.dt.bfloat16
USE_BF16 = os.environ.get("KERNEL_DT", "bf16") == "bf16"
DT_MM = BF16 if USE_BF16 else F32R
NP_MM = ml_dtypes.bfloat16 if USE_BF16 else np.float32
AF = mybir.ActivationFunctionType
OP = mybir.AluOpType

_NC_CACHE = []


def _build_nc():
    nc = bacc.Bacc(num_devices=NCORES)

    # ---- I/O ----
    img_p = nc.declare_dram_parameter("img", [S, CT, 128, L + 2], F32, isOutput=False)
    imm_p = nc.declare_dram_parameter("imm", [S, CT, 128, L + 2], DT_MM, isOutput=False)
    cli_p = nc.declare_dram_parameter("cli", [S, 3, L], DT_MM, isOutput=False)
    w_cc_p = {}   # 256->256 conv weights, [CT(kt), 128(ki), 768=(t,mt)*128+mi]
    for name in ("wq_a", "wo_a", "wk_b", "wo_b"):
        w_cc_p[name] = nc.declare_dram_parameter(name, [CT, 128, 768], DT_MM, isOutput=False)
    wv_b_p = nc.declare_dram_parameter("wv_b", [CT, 3, 128, C], DT_MM, isOutput=False)
    w_sm_p = {}   # 1->256 convs as [3, 256]
    for name in ("wk_a", "wv_a", "wq_b"):
        w_sm_p[name] = nc.declare_dram_parameter(name, [3, C], DT_MM, isOutput=False)
    bias_p = {}
    for name in ("qb_a", "kb_a", "ob_a", "qb_b", "kb_b", "ob_b"):
        bias_p[name] = nc.declare_dram_parameter(name, [128, CT], F32, isOutput=False)
    vb_p = {}
    for name in ("vb_a", "vb_b"):
        vb_p[name] = nc.declare_dram_parameter(name, [1, C], F32R, isOutput=False)
    gb_p = {}
    for name in ("ga_a", "be_a", "ga_b", "be_b"):
        gb_p[name] = nc.declare_dram_parameter(name, [128, CT], F32, isOutput=False)
    out_p = nc.declare_dram_parameter("out", [S, 2, CT, 128, L], F32, isOutput=True)

    from contextlib import ExitStack
    with tile.TileContext(nc) as tc, ExitStack() as es:
        ec = es.enter_context
        wgt = ec(tc.tile_pool(name="wgt", bufs=1))
        io = ec(tc.tile_pool(name="io", bufs=1))
        qk = ec(tc.tile_pool(name="qk", bufs=2))
        vtp = ec(tc.tile_pool(name="vtp", bufs=2))
        ex = ec(tc.tile_pool(name="ex", bufs=3))
        cx = ec(tc.tile_pool(name="cx", bufs=2))
        op_pool = ec(tc.tile_pool(name="op", bufs=1))
        sm = ec(tc.tile_pool(name="sm", bufs=2))
        st = ec(tc.tile_pool(name="st", bufs=1))
        bn = ec(tc.tile_pool(name="bn", bufs=3))
        dram = ec(tc.tile_pool(name="dram", bufs=1, space="DRAM"))
        ps_conv = ec(tc.tile_pool(name="psc", bufs=2, space="PSUM"))
        ps_sc = ec(tc.tile_pool(name="pss", bufs=2, space="PSUM"))
        ps_ctx = ec(tc.tile_pool(name="psx", bufs=1, space="PSUM"))
        ps_den = ec(tc.tile_pool(name="psd", bufs=1, space="PSUM"))
        ps_bc = ec(tc.tile_pool(name="psb", bufs=1, space="PSUM"))
        if True:
            # ---- load weights/constants to SBUF ----
            w_cc = {}
            for name in ("wq_a", "wo_a", "wk_b", "wo_b"):
                w_cc[name] = []
                for kt in range(CT):
                    t_ = wgt.tile([128, 768], DT_MM, tag=f"{name}_{kt}")
                    nc.sync.dma_start(out=t_, in_=w_cc_p[name][kt])
                    w_cc[name].append(t_)
            wv_b = []
            for kt in range(CT):
                row = []
                for t in range(3):
                    t_ = wgt.tile([128, C], DT_MM, tag=f"wv_b_{kt}_{t}")
                    nc.sync.dma_start(out=t_, in_=wv_b_p[kt, t])
                    row.append(t_)
                wv_b.append(row)
            w_sm = {}
            for name in ("wk_a", "wv_a", "wq_b"):
                t_ = wgt.tile([3, C], DT_MM, tag=name)
                nc.sync.dma_start(out=t_, in_=w_sm_p[name][:, :])
                w_sm[name] = t_
            bias = {}
            for name in ("qb_a", "kb_a", "ob_a", "qb_b", "kb_b", "ob_b"):
                t_ = wgt.tile([128, CT], F32, tag=name)
                nc.sync.dma_start(out=t_, in_=bias_p[name][:, :])
                bias[name] = t_
            vb = {}
            for name in ("vb_a", "vb_b"):
                t_ = wgt.tile([1, C], F32R, tag=name)
                nc.sync.dma_start(out=t_, in_=vb_p[name][:, :])
                vb[name] = t_
            gb = {}
            for name in ("ga_a", "be_a", "ga_b", "be_b"):
                t_ = wgt.tile([128, CT], F32, tag=name)
                nc.sync.dma_start(out=t_, in_=gb_p[name][:, :])
                gb[name] = t_
            ones_f32c = wgt.tile([128, 1], F32, tag="ones_f32c")
            nc.vector.memset(ones_f32c, 1.0)
            ones_col = wgt.tile([128, 1], DT_MM, tag="ones_col")
            nc.vector.tensor_copy(out=ones_col, in_=ones_f32c)
            ones_f32r_ = wgt.tile([1, 128], F32, tag="ones_f32r_")
            nc.vector.memset(ones_f32r_, 1.0)
            ones_row = wgt.tile([1, 128], F32R, tag="ones_row")
            nc.vector.tensor_copy(out=ones_row, in_=ones_f32r_)
            eps_sb = wgt.tile([128, 1], F32, tag="eps_sb")
            nc.vector.memset(eps_sb, EPS)
            zero_col = wgt.tile([128, 1], F32, tag="zero_col")
            nc.vector.memset(zero_col, 0.0)

            img = []   # [s][kt] -> [128, L+2] f32 (residual)
            imm = []   # [s][kt] -> [128, L+2] matmul dtype
            cli = []   # [s] -> [3, L]
            for s in range(S):
                img.append([])
                imm.append([])
                for kt in range(CT):
                    t_ = io.tile([128, L + 2], F32, tag=f"img_{s}_{kt}")
                    nc.sync.dma_start(out=t_, in_=img_p[s, kt])
                    img[s].append(t_)
                    t_ = io.tile([128, L + 2], DT_MM, tag=f"imm_{s}_{kt}")
                    nc.sync.dma_start(out=t_, in_=imm_p[s, kt])
                    imm[s].append(t_)
                t_ = io.tile([3, L], DT_MM, tag=f"cli_{s}")
                nc.sync.dma_start(out=t_, in_=cli_p[s])
                cli.append(t_)

            # ---- helpers ----
            def conv_cc(dst, w_kt, bias_ap, src, stats=None):
                """256->256 K=3 conv: dst[ct][:, l] from padded src[kt] tiles."""
                for ct in range(CT):
                    for ls in range(LS):
                        p = ps_conv.tile([128, 512], F32, tag="conv", name="convp")
                        n = 0
                        for kt in range(CT):
                            for t in range(3):
                                nc.tensor.matmul(
                                    p,
                                    lhsT=w_kt[kt][:, (t * 2 + ct) * 128:(t * 2 + ct + 1) * 128],
                                    rhs=src[kt][:, ls * 512 + t: ls * 512 + t + 512],
                                    start=(n == 0), stop=(n == 5))
                                n += 1
                        acc = None if stats is None else stats(ct, ls)
                        nc.scalar.activation(
                            out=dst[ct][:, ls * 512:(ls + 1) * 512], in_=p,
                            func=AF.Identity, bias=bias_ap[:, ct:ct + 1], scale=1.0,
                            accum_out=acc)

            def conv_1c(dst, w_lhsT, bias_ap, cli_t):
                """1->256 K=3 conv via [3,*] im2col rows."""
                for ct in range(CT):
                    for ls in range(LS):
                        p = ps_conv.tile([128, 512], F32, tag="conv", name="convp")
                        nc.tensor.matmul(
                            p, lhsT=w_lhsT[:, ct * 128:(ct + 1) * 128],
                            rhs=cli_t[:, ls * 512:(ls + 1) * 512],
                            start=True, stop=True)
                        nc.scalar.activation(
                            out=dst[ct][:, ls * 512:(ls + 1) * 512], in_=p,
                            func=AF.Identity, bias=bias_ap[:, ct:ct + 1], scale=1.0)

            o_tiles = {}
            slots = {}
            statg = {}

            for br in range(2):
                abr = "a" if br == 0 else "b"
                slots[br] = st.tile([128, 4 * S * LS], F32, tag=f"slots{br}", name=f"slots{br}")
                for s in range(S):
                    # ---- convs ----
                    q_sb = [qk.tile([128, L], DT_MM, tag=f"q{kt}", name=f"q{kt}") for kt in range(CT)]
                    k_sb = [qk.tile([128, L], DT_MM, tag=f"k{kt}", name=f"k{kt}") for kt in range(CT)]
                    vt = [vtp.tile([128, C], DT_MM, tag=f"vt{mt}", name=f"vt{mt}") for mt in range(8)]
                    if br == 0:
                        conv_cc(q_sb, w_cc["wq_a"], bias["qb_a"], imm[s])
                        conv_1c(k_sb, w_sm["wk_a"], bias["kb_a"], cli[s])
                        for mt in range(8):
                            p = ps_conv.tile([128, C], F32, tag="conv", name="convp")
                            nc.tensor.matmul(
                                p, lhsT=cli[s][:, mt * 128:(mt + 1) * 128],
                                rhs=w_sm["wv_a"], start=True, stop=True)
                            nc.vector.tensor_copy(out=vt[mt], in_=p)
                    else:
                        conv_1c(q_sb, w_sm["wq_b"], bias["qb_b"], cli[s])
                        conv_cc(k_sb, w_cc["wk_b"], bias["kb_b"], imm[s])
                        for mt in range(8):
                            p = ps_conv.tile([128, C], F32, tag="conv", name="convp")
                            n = 0
                            for kt in range(CT):
                                for t in range(3):
                                    nc.tensor.matmul(
                                        p,
                                        lhsT=imm[s][kt][:, mt * 128 + t: mt * 128 + t + 128],
                                        rhs=wv_b[kt][t],
                                        start=(n == 0), stop=(n == 5))
                                    n += 1
                            nc.vector.tensor_copy(out=vt[mt], in_=p)

                    # ---- attention (m-loop software-pipelined: scores for
                    # step mt+1 issue on PE before ctx of step mt, so the PE
                    # stays busy while ScalarE computes exp of step mt) ----
                    ctx = [cx.tile([128, L + 2], DT_MM, tag=f"ctx{kt}", name=f"ctx{kt}") for kt in range(CT)]
                    for ct in range(CT):
                        nc.vector.tensor_copy(out=ctx[ct][:, 0:1], in_=zero_col)
                        nc.vector.tensor_copy(out=ctx[ct][:, L + 1:L + 2], in_=zero_col)
                    for ls in range(LS):
                        ctx_ps = [ps_ctx.tile([128, 512], F32, tag=f"ctxp{ct}", name=f"ctxp{ct}") for ct in range(CT)]
                        den_ps = ps_den.tile([1, 512], F32, tag="den", name="den_ps")
                        ets = {}

                        def _sc_exp(mt):
                            sc = ps_sc.tile([128, 512], F32, tag="sc", name="sc")
                            for kt in range(CT):
                                nc.tensor.matmul(
                                    sc, lhsT=k_sb[kt][:, mt * 128:(mt + 1) * 128],
                                    rhs=q_sb[kt][:, ls * 512:(ls + 1) * 512],
                                    start=(kt == 0), stop=(kt == CT - 1))
                            et = ex.tile([128, 512], DT_MM, tag="expT", name="et")
                            nc.scalar.activation(out=et, in_=sc, func=AF.Exp, scale=SCALE)
                            ets[mt] = et

                        _sc_exp(0)
                        _sc_exp(1)
                        for mt in range(8):
                            if mt + 2 < 8:
                                _sc_exp(mt + 2)
                            et = ets.pop(mt)
                            for ct in range(CT):
                                nc.tensor.matmul(
                                    ctx_ps[ct], lhsT=vt[mt][:, ct * 128:(ct + 1) * 128],
                                    rhs=et, start=(mt == 0), stop=False)
                            nc.tensor.matmul(
                                den_ps, lhsT=ones_col, rhs=et,
                                start=(mt == 0), stop=(mt == 7))
                        den_sb = sm.tile([1, 512], F32R, tag="den_sb")
                        nc.vector.tensor_copy(out=den_sb, in_=den_ps)
                        for ct in range(CT):
                            nc.tensor.matmul(
                                ctx_ps[ct], lhsT=vb[f"vb_{abr}"][:, ct * 128:(ct + 1) * 128],
                                rhs=den_sb, start=False, stop=True)
                        # broadcast den across partitions, then wide reciprocal
                        bc_ps = ps_bc.tile([128, 512], F32, tag="bc", name="bc_ps")
                        nc.tensor.matmul(bc_ps, lhsT=ones_row, rhs=den_sb, start=True, stop=True)
                        bc_sb = sm.tile([128, 512], F32, tag="bc_sb")
                        nc.vector.reciprocal(out=bc_sb, in_=bc_ps)
                        for ct in range(CT):
                            nc.vector.tensor_mul(
                                out=ctx[ct][:, 1 + ls * 512: 1 + (ls + 1) * 512],
                                in0=ctx_ps[ct], in1=bc_sb)

                    # ---- out conv + stats ----
                    o_sb = [op_pool.tile([128, L], F32, tag=f"o_{br}_{s}_{ct}", name=f"o_{br}_{s}_{ct}") for ct in range(CT)]
                    for ct in range(CT):
                        o_tiles[(br, s, ct)] = o_sb[ct]

                    def _acc(ct, ls, _br=br, _s=s):
                        i = ct * S * LS + _s * LS + ls
                        return slots[_br][:, i:i + 1]

                    conv_cc(o_sb, w_cc[f"wo_{abr}"], bias[f"ob_{abr}"], ctx, stats=_acc)
                    for ct in range(CT):
                        for ls in range(LS):
                            sq = sm.tile([128, 512], F32, tag="sqscr", name="sq")
                            osl = o_sb[ct][:, ls * 512:(ls + 1) * 512]
                            nc.vector.tensor_mul(out=sq, in0=osl, in1=osl)
                            i = (2 + ct) * S * LS + s * LS + ls
                            nc.vector.reduce_sum(
                                out=slots[br][:, i:i + 1], in_=sq,
                                axis=mybir.AxisListType.X)

                # ---- cross-core stats all-reduce for this branch ----
                statp = st.tile([128, 4], F32, tag=f"statp{br}")
                nc.vector.reduce_sum(out=statp, in_=slots[br].rearrange("p (g i) -> p g i", i=S * LS), axis=mybir.AxisListType.X)
                cc_in = dram.tile([128, 4], F32, tag=f"ccin{br}")
                cc_out = dram.tile([128, 4], F32, tag=f"ccout{br}")
                nc.sync.dma_start(out=cc_in, in_=statp)
                if os.environ.get("KERNEL_NO_CC"):
                    nc.sync.dma_start(out=cc_out, in_=cc_in)
                else:
                    nc.gpsimd.collective_compute(
                        "AllReduce", OP.add,
                        replica_groups=[list(range(NCORES))],
                        ins=[cc_in.opt()], outs=[cc_out.opt()])
                sg = st.tile([128, 4], F32, tag=f"statg{br}")
                nc.sync.dma_start(out=sg, in_=cc_out)
                statg[br] = sg

            # ---- BN finalize + residual + output ----
            for br in range(2):
                abr = "a" if br == 0 else "b"
                sg = statg[br]
                mean = st.tile([128, CT], F32, tag=f"mean{br}")
                nc.vector.tensor_scalar_mul(mean, sg[:, 0:2], 1.0 / NSTAT)
                esq = st.tile([128, CT], F32, tag=f"esq{br}")
                nc.vector.tensor_scalar_mul(esq, sg[:, 2:4], 1.0 / NSTAT)
                m2 = st.tile([128, CT], F32, tag=f"m2{br}")
                nc.vector.tensor_mul(out=m2, in0=mean, in1=mean)
                var = st.tile([128, CT], F32, tag=f"var{br}")
                nc.vector.tensor_sub(out=var, in0=esq, in1=m2)
                sd = st.tile([128, CT], F32, tag=f"sd{br}")
                nc.scalar.activation(out=sd, in_=var, func=AF.Sqrt, bias=eps_sb[:, 0:1], scale=1.0)
                rstd = st.tile([128, CT], F32, tag=f"rstd{br}")
                nc.vector.reciprocal(out=rstd, in_=sd)
                A_ = st.tile([128, CT], F32, tag=f"A{br}")
                nc.vector.tensor_mul(out=A_, in0=rstd, in1=gb[f"ga_{abr}"])
                mA = st.tile([128, CT], F32, tag=f"mA{br}")
                nc.vector.tensor_mul(out=mA, in0=mean, in1=A_)
                Bc = st.tile([128, CT], F32, tag=f"Bc{br}")
                nc.vector.tensor_sub(out=Bc, in0=gb[f"be_{abr}"], in1=mA)
                for s in range(S):
                    for ct in range(CT):
                        for ls in range(LS):
                            tmp = bn.tile([128, 512], F32, tag="bnt", name="bnt")
                            nc.scalar.activation(
                                out=tmp,
                                in_=o_tiles[(br, s, ct)][:, ls * 512:(ls + 1) * 512],
                                func=AF.Identity,
                                scale=A_[:, ct:ct + 1], bias=Bc[:, ct:ct + 1])
                            res = bn.tile([128, 512], F32, tag="bnr", name="bnr")
                            nc.vector.tensor_add(
                                out=res, in0=tmp,
                                in1=img[s][ct][:, 1 + ls * 512: 1 + (ls + 1) * 512])
                            nc.sync.dma_start(
                                out=out_p[s, br, ct, :, ls * 512:(ls + 1) * 512], in_=res)

    nc.compile()
    return nc


def _get_nc():
    if not _NC_CACHE:
        _NC_CACHE.append(_build_nc())
    return _NC_CACHE[0]


def _prep_shared(inp):
    f = NP_MM
    m = {}

    def cc_layout(w):
        # [256(mt*128+mi), 256(kt*128+ki), 3(t)] -> [kt, ki, 768=(t*2+mt)*128+mi]
        return np.ascontiguousarray(
            np.asarray(w).reshape(2, 128, 2, 128, 3).transpose(2, 3, 4, 0, 1).reshape(2, 128, 768)).astype(f)

    m["wq_a"] = cc_layout(inp["a_qw"])
    m["wo_a"] = cc_layout(inp["a_ow"])
    m["wk_b"] = cc_layout(inp["b_kw"])
    m["wo_b"] = cc_layout(inp["b_ow"])
    # b_vw [c, c', t] -> [kt, t, ki, c]
    m["wv_b"] = np.ascontiguousarray(
        np.asarray(inp["b_vw"]).reshape(C, 2, 128, 3).transpose(1, 3, 2, 0)).astype(f)
    m["wk_a"] = np.ascontiguousarray(np.asarray(inp["a_kw"])[:, 0, :].T).astype(f)
    m["wv_a"] = np.ascontiguousarray(np.asarray(inp["a_vw"])[:, 0, :].T).astype(f)
    m["wq_b"] = np.ascontiguousarray(np.asarray(inp["b_qw"])[:, 0, :].T).astype(f)
    for dst, src in (("qb_a", "a_qb"), ("kb_a", "a_kb"), ("ob_a", "a_ob"),
                     ("qb_b", "b_qb"), ("kb_b", "b_kb"), ("ob_b", "b_ob"),
                     ("ga_a", "a_g"), ("be_a", "a_beta"),
                     ("ga_b", "b_g"), ("be_b", "b_beta")):
        m[dst] = np.ascontiguousarray(np.asarray(inp[src]).reshape(2, 128).T).astype(np.float32)
    m["vb_a"] = np.ascontiguousarray(np.asarray(inp["a_vb"])[None, :]).astype(np.float32)
    m["vb_b"] = np.ascontiguousarray(np.asarray(inp["b_vb"])[None, :]).astype(np.float32)
    return m


def _core_maps(image, clinical, shared, ncores=NCORES):
    in_maps = []
    for core in range(ncores):
        m = dict(shared)
        sl = slice(core * S, (core + 1) * S)
        a = image[sl].reshape(S, CT, 128, L)
        pad = np.zeros((S, CT, 128, L + 2), np.float32)
        pad[..., 1:L + 1] = a
        m["img"] = pad
        m["imm"] = pad.astype(NP_MM)
        c = clinical[sl][:, 0, :]
        im2 = np.zeros((S, 3, L), np.float32)
        im2[:, 0, 1:] = c[:, :L - 1]
        im2[:, 1, :] = c
        im2[:, 2, :L - 1] = c[:, 1:]
        m["cli"] = im2.astype(NP_MM)
        in_maps.append(m)
    return in_maps


def kernel(**inputs):
    inp = {k: np.asarray(v) for k, v in inputs.items()}
    nc = _get_nc()
    shared = _prep_shared(inp)
    image = inp["image"].astype(np.float32)
    clinical = inp["clinical"].astype(np.float32)
    in_maps = _core_maps(image, clinical, shared)
    res = run_bass_kernel_spmd(nc, in_maps, core_ids=list(range(NCORES)))
    outs = np.concatenate([res.results[i]["out"] for i in range(NCORES)], axis=0)
    return np.ascontiguousarray(outs.reshape(16, 512, L))



# revision 8
# speedup vs baseline: 1.0935x; 1.0935x over previous
"""CrossSymmetricModal trn2 kernel: 2 cross-attention branches + BN + residual.

Data-parallel over batch (2 samples/core on 8 cores); BatchNorm batch stats
cross-core via a small AllReduce per branch.

Heavy matmuls run in bf16 on the TensorEngine (fp32 PSUM accumulation):
 - convs (K=3, 'same' pad) are tap-shifted matmuls accumulating in PSUM; the
   1->256 convs contract over the 3-row im2col of clinical.
 - attention is transpose-free: scoresT[m,l] = k^T q is produced directly in
   [m (partition), l] orientation, exp on ScalarE (no max subtraction: scores
   are bounded ~|1.6|), the softmax denominator is a ones-vector matmul, and
   ctx[c,l] = vT.T @ expT needs vT which each branch's v-conv emits directly
   in transposed [m, c] layout.
The residual image, all reductions and the BN math stay in fp32.
"""
import os
import sys

sys.path.insert(0, '/opt/trn_rl_repo')

import ml_dtypes
import numpy as np

from concourse import bacc, mybir, tile
from concourse.bass_utils import run_bass_kernel_spmd

S = 2            # samples per core
NCORES = 8
C = 256
CT = 2           # 128-partition channel tiles
L = 1024
LS = 2           # 512-wide l slices
EPS = 1e-5
SCALE = 1.0 / 16.0   # 1/sqrt(C)
NSTAT = 16 * L       # batchnorm reduction size (full batch x length)

F32 = mybir.dt.float32
F32R = mybir.dt.float32r
BF16 = mybir.dt.bfloat16
USE_BF16 = os.environ.get("KERNEL_DT", "bf16") == "bf16"
DT_MM = BF16 if USE_BF16 else F32R
NP_MM = ml_dtypes.bfloat16 if USE_BF16 else np.float32
AF = mybir.ActivationFunctionType
OP = mybir.AluOpType

_NC_CACHE = []


def _build_nc():
    nc = bacc.Bacc(num_devices=NCORES)

    # ---- I/O ----
    img_p = nc.declare_dram_parameter("img", [S, CT, 128, L + 2], F32, isOutput=False)
    imm_p = nc.declare_dram_parameter("imm", [S, CT, 128, L + 2], DT_MM, isOutput=False)
    cli_p = nc.declare_dram_parameter("cli", [S, 3, L], DT_MM, isOutput=False)
    w_cc_p = {}   # 256->256 conv weights, [CT(kt), 128(ki), 768=(t,mt)*128+mi]
    for name in ("wq_a", "wo_a", "wk_b", "wo_b"):
        w_cc_p[name] = nc.declare_dram_parameter(name, [CT, 128, 768], DT_MM, isOutput=False)
    wv_b_p = nc.declare_dram_parameter("wv_b", [CT, 3, 128, C], DT_MM, isOutput=False)
    w_sm_p = {}   # 1->256 convs as [3, 256]
    for name in ("wk_a", "wv_a", "wq_b"):
        w_sm_p[name] = nc.declare_dram_parameter(name, [3, C], DT_MM, isOutput=False)
    bias_p = {}
    for name in ("qb_a", "kb_a", "ob_a", "qb_b", "kb_b", "ob_b"):
        bias_p[name] = nc.declare_dram_parameter(name, [128, CT], F32, isOutput=False)
    vb_p = {}
    for name in ("vb_a", "vb_b"):
        vb_p[name] = nc.declare_dram_parameter(name, [1, C], F32R, isOutput=False)
    gb_p = {}
    for name in ("ga_a", "be_a", "ga_b", "be_b"):
        gb_p[name] = nc.declare_dram_parameter(name, [128, CT], F32, isOutput=False)
    out_p = nc.declare_dram_parameter("out", [S, 2, CT, 128, L], F32, isOutput=True)

    from contextlib import ExitStack
    with tile.TileContext(nc) as tc, ExitStack() as es:
        ec = es.enter_context
        wgt = ec(tc.tile_pool(name="wgt", bufs=1))
        io = ec(tc.tile_pool(name="io", bufs=1))
        qk = ec(tc.tile_pool(name="qk", bufs=2))
        vtp = ec(tc.tile_pool(name="vtp", bufs=2))
        ex = ec(tc.tile_pool(name="ex", bufs=3))
        cx = ec(tc.tile_pool(name="cx", bufs=2))
        op_pool = ec(tc.tile_pool(name="op", bufs=1))
        sm = ec(tc.tile_pool(name="sm", bufs=2))
        st = ec(tc.tile_pool(name="st", bufs=1))
        bn = ec(tc.tile_pool(name="bn", bufs=3))
        dram = ec(tc.tile_pool(name="dram", bufs=1, space="DRAM"))
        ps_conv = ec(tc.tile_pool(name="psc", bufs=2, space="PSUM"))
        ps_sc = ec(tc.tile_pool(name="pss", bufs=2, space="PSUM"))
        ps_ctx = ec(tc.tile_pool(name="psx", bufs=1, space="PSUM"))
        ps_den = ec(tc.tile_pool(name="psd", bufs=1, space="PSUM"))
        ps_bc = ec(tc.tile_pool(name="psb", bufs=1, space="PSUM"))
        if True:
            # ---- load weights/constants to SBUF ----
            # DMA issue order matters: the first conv (branch a, s0) needs
            # wq_a + qb_a + imm[0] only, so those go first; the residual img
            # tiles are needed only at BN-finalize and stream last.
            w_cc = {name: [] for name in ("wq_a", "wo_a", "wk_b", "wo_b")}
            for kt in range(CT):
                t_ = wgt.tile([128, 768], DT_MM, tag=f"wq_a_{kt}")
                nc.sync.dma_start(out=t_, in_=w_cc_p["wq_a"][kt])
                w_cc["wq_a"].append(t_)
            bias = {}
            for name in ("qb_a", "kb_a", "ob_a", "qb_b", "kb_b", "ob_b"):
                t_ = wgt.tile([128, CT], F32, tag=name)
                nc.sync.dma_start(out=t_, in_=bias_p[name][:, :])
                bias[name] = t_
            imm = []   # [s][kt] -> [128, L+2] matmul dtype
            cli = []   # [s] -> [3, L]
            for s in range(S):
                imm.append([])
                for kt in range(CT):
                    t_ = io.tile([128, L + 2], DT_MM, tag=f"imm_{s}_{kt}")
                    nc.sync.dma_start(out=t_, in_=imm_p[s, kt])
                    imm[s].append(t_)
                t_ = io.tile([3, L], DT_MM, tag=f"cli_{s}")
                nc.sync.dma_start(out=t_, in_=cli_p[s])
                cli.append(t_)
            w_sm = {}
            for name in ("wk_a", "wv_a", "wq_b"):
                t_ = wgt.tile([3, C], DT_MM, tag=name)
                nc.sync.dma_start(out=t_, in_=w_sm_p[name][:, :])
                w_sm[name] = t_
            vb = {}
            for name in ("vb_a", "vb_b"):
                t_ = wgt.tile([1, C], F32R, tag=name)
                nc.sync.dma_start(out=t_, in_=vb_p[name][:, :])
                vb[name] = t_
            for name in ("wo_a", "wk_b", "wo_b"):
                for kt in range(CT):
                    t_ = wgt.tile([128, 768], DT_MM, tag=f"{name}_{kt}")
                    nc.sync.dma_start(out=t_, in_=w_cc_p[name][kt])
                    w_cc[name].append(t_)
            wv_b = []
            for kt in range(CT):
                row = []
                for t in range(3):
                    t_ = wgt.tile([128, C], DT_MM, tag=f"wv_b_{kt}_{t}")
                    nc.sync.dma_start(out=t_, in_=wv_b_p[kt, t])
                    row.append(t_)
                wv_b.append(row)
            gb = {}
            for name in ("ga_a", "be_a", "ga_b", "be_b"):
                t_ = wgt.tile([128, CT], F32, tag=name)
                nc.sync.dma_start(out=t_, in_=gb_p[name][:, :])
                gb[name] = t_
            img = []   # [s][kt] -> [128, L+2] f32 (residual)
            for s in range(S):
                img.append([])
                for kt in range(CT):
                    t_ = io.tile([128, L + 2], F32, tag=f"img_{s}_{kt}")
                    nc.sync.dma_start(out=t_, in_=img_p[s, kt])
                    img[s].append(t_)
            ones_f32c = wgt.tile([128, 1], F32, tag="ones_f32c")
            nc.vector.memset(ones_f32c, 1.0)
            ones_col = wgt.tile([128, 1], DT_MM, tag="ones_col")
            nc.vector.tensor_copy(out=ones_col, in_=ones_f32c)
            ones_f32r_ = wgt.tile([1, 128], F32, tag="ones_f32r_")
            nc.vector.memset(ones_f32r_, 1.0)
            ones_row = wgt.tile([1, 128], F32R, tag="ones_row")
            nc.vector.tensor_copy(out=ones_row, in_=ones_f32r_)
            eps_sb = wgt.tile([128, 1], F32, tag="eps_sb")
            nc.vector.memset(eps_sb, EPS)
            zero_col = wgt.tile([128, 1], F32, tag="zero_col")
            nc.vector.memset(zero_col, 0.0)

            # ---- helpers ----
            def conv_cc(dst, w_kt, bias_ap, src, stats=None):
                """256->256 K=3 conv: dst[ct][:, l] from padded src[kt] tiles."""
                for ct in range(CT):
                    for ls in range(LS):
                        p = ps_conv.tile([128, 512], F32, tag="conv", name="convp")
                        n = 0
                        for kt in range(CT):
                            for t in range(3):
                                nc.tensor.matmul(
                                    p,
                                    lhsT=w_kt[kt][:, (t * 2 + ct) * 128:(t * 2 + ct + 1) * 128],
                                    rhs=src[kt][:, ls * 512 + t: ls * 512 + t + 512],
                                    start=(n == 0), stop=(n == 5))
                                n += 1
                        acc = None if stats is None else stats(ct, ls)
                        nc.scalar.activation(
                            out=dst[ct][:, ls * 512:(ls + 1) * 512], in_=p,
                            func=AF.Identity, bias=bias_ap[:, ct:ct + 1], scale=1.0,
                            accum_out=acc)

            def conv_1c(dst, w_lhsT, bias_ap, cli_t):
                """1->256 K=3 conv via [3,*] im2col rows."""
                for ct in range(CT):
                    for ls in range(LS):
                        p = ps_conv.tile([128, 512], F32, tag="conv", name="convp")
                        nc.tensor.matmul(
                            p, lhsT=w_lhsT[:, ct * 128:(ct + 1) * 128],
                            rhs=cli_t[:, ls * 512:(ls + 1) * 512],
                            start=True, stop=True)
                        nc.scalar.activation(
                            out=dst[ct][:, ls * 512:(ls + 1) * 512], in_=p,
                            func=AF.Identity, bias=bias_ap[:, ct:ct + 1], scale=1.0)

            o_tiles = {}
            slots = {}
            statg = {}

            for br in range(2):
                abr = "a" if br == 0 else "b"
                slots[br] = st.tile([128, 4 * S * LS], F32, tag=f"slots{br}", name=f"slots{br}")
                for s in range(S):
                    # ---- convs ----
                    q_sb = [qk.tile([128, L], DT_MM, tag=f"q{kt}", name=f"q{kt}") for kt in range(CT)]
                    k_sb = [qk.tile([128, L], DT_MM, tag=f"k{kt}", name=f"k{kt}") for kt in range(CT)]
                    vt = [vtp.tile([128, C], DT_MM, tag=f"vt{mt}", name=f"vt{mt}") for mt in range(8)]
                    if br == 0:
                        conv_cc(q_sb, w_cc["wq_a"], bias["qb_a"], imm[s])
                        conv_1c(k_sb, w_sm["wk_a"], bias["kb_a"], cli[s])
                        for mt in range(8):
                            p = ps_conv.tile([128, C], F32, tag="conv", name="convp")
                            nc.tensor.matmul(
                                p, lhsT=cli[s][:, mt * 128:(mt + 1) * 128],
                                rhs=w_sm["wv_a"], start=True, stop=True)
                            nc.vector.tensor_copy(out=vt[mt], in_=p)
                    else:
                        conv_1c(q_sb, w_sm["wq_b"], bias["qb_b"], cli[s])
                        conv_cc(k_sb, w_cc["wk_b"], bias["kb_b"], imm[s])
                        for mt in range(8):
                            p = ps_conv.tile([128, C], F32, tag="conv", name="convp")
                            n = 0
                            for kt in range(CT):
                                for t in range(3):
                                    nc.tensor.matmul(
                                        p,
                                        lhsT=imm[s][kt][:, mt * 128 + t: mt * 128 + t + 128],
                                        rhs=wv_b[kt][t],
                                        start=(n == 0), stop=(n == 5))
                                    n += 1
                            nc.vector.tensor_copy(out=vt[mt], in_=p)

                    # ---- attention (m-loop software-pipelined: scores for
                    # step mt+1 issue on PE before ctx of step mt, so the PE
                    # stays busy while ScalarE computes exp of step mt) ----
                    ctx = [cx.tile([128, L + 2], DT_MM, tag=f"ctx{kt}", name=f"ctx{kt}") for kt in range(CT)]
                    for ct in range(CT):
                        nc.vector.tensor_copy(out=ctx[ct][:, 0:1], in_=zero_col)
                        nc.vector.tensor_copy(out=ctx[ct][:, L + 1:L + 2], in_=zero_col)
                    for ls in range(LS):
                        ctx_ps = [ps_ctx.tile([128, 512], F32, tag=f"ctxp{ct}", name=f"ctxp{ct}") for ct in range(CT)]
                        den_ps = ps_den.tile([1, 512], F32, tag="den", name="den_ps")
                        ets = {}

                        def _sc_exp(mt):
                            sc = ps_sc.tile([128, 512], F32, tag="sc", name="sc")
                            for kt in range(CT):
                                nc.tensor.matmul(
                                    sc, lhsT=k_sb[kt][:, mt * 128:(mt + 1) * 128],
                                    rhs=q_sb[kt][:, ls * 512:(ls + 1) * 512],
                                    start=(kt == 0), stop=(kt == CT - 1))
                            et = ex.tile([128, 512], DT_MM, tag="expT", name="et")
                            nc.scalar.activation(out=et, in_=sc, func=AF.Exp, scale=SCALE)
                            ets[mt] = et

                        _sc_exp(0)
                        _sc_exp(1)
                        for mt in range(8):
                            if mt + 2 < 8:
                                _sc_exp(mt + 2)
                            et = ets.pop(mt)
                            for ct in range(CT):
                                nc.tensor.matmul(
                                    ctx_ps[ct], lhsT=vt[mt][:, ct * 128:(ct + 1) * 128],
                                    rhs=et, start=(mt == 0), stop=False)
                            nc.tensor.matmul(
                                den_ps, lhsT=ones_col, rhs=et,
                                start=(mt == 0), stop=(mt == 7))
                        den_sb = sm.tile([1, 512], F32R, tag="den_sb")
                        nc.vector.tensor_copy(out=den_sb, in_=den_ps)
                        for ct in range(CT):
                            nc.tensor.matmul(
                                ctx_ps[ct], lhsT=vb[f"vb_{abr}"][:, ct * 128:(ct + 1) * 128],
                                rhs=den_sb, start=False, stop=True)
                        # 1/den on the single row (den ~1e2..1e3, no edge
                        # cases), then broadcast via PE; multiply from PSUM
                        den_f = sm.tile([1, 512], F32, tag="den_f")
                        nc.vector.tensor_copy(out=den_f, in_=den_ps)
                        rden = sm.tile([1, 512], F32, tag="rden")
                        nc.vector.reciprocal_approx_fast(out=rden, in_=den_f)
                        rden_r = sm.tile([1, 512], F32R, tag="rden_r")
                        nc.vector.tensor_copy(out=rden_r, in_=rden)
                        bc_ps = ps_bc.tile([128, 512], F32, tag="bc", name="bc_ps")
                        nc.tensor.matmul(bc_ps, lhsT=ones_row, rhs=rden_r, start=True, stop=True)
                        bc_sb = sm.tile([128, 512], F32, tag="bc_sb")
                        nc.vector.tensor_copy(out=bc_sb, in_=bc_ps)
                        for ct in range(CT):
                            nc.vector.tensor_mul(
                                out=ctx[ct][:, 1 + ls * 512: 1 + (ls + 1) * 512],
                                in0=ctx_ps[ct], in1=bc_sb)

                    # ---- out conv + stats ----
                    o_sb = [op_pool.tile([128, L], F32, tag=f"o_{br}_{s}_{ct}", name=f"o_{br}_{s}_{ct}") for ct in range(CT)]
                    for ct in range(CT):
                        o_tiles[(br, s, ct)] = o_sb[ct]

                    def _acc(ct, ls, _br=br, _s=s):
                        i = ct * S * LS + _s * LS + ls
                        return slots[_br][:, i:i + 1]

                    conv_cc(o_sb, w_cc[f"wo_{abr}"], bias[f"ob_{abr}"], ctx, stats=_acc)
                    for ct in range(CT):
                        for ls in range(LS):
                            sq = sm.tile([128, 512], F32, tag="sqscr", name="sq")
                            osl = o_sb[ct][:, ls * 512:(ls + 1) * 512]
                            nc.vector.tensor_mul(out=sq, in0=osl, in1=osl)
                            i = (2 + ct) * S * LS + s * LS + ls
                            nc.vector.reduce_sum(
                                out=slots[br][:, i:i + 1], in_=sq,
                                axis=mybir.AxisListType.X)

                # ---- cross-core stats all-reduce for this branch ----
                statp = st.tile([128, 4], F32, tag=f"statp{br}")
                nc.vector.reduce_sum(out=statp, in_=slots[br].rearrange("p (g i) -> p g i", i=S * LS), axis=mybir.AxisListType.X)
                cc_in = dram.tile([128, 4], F32, tag=f"ccin{br}")
                cc_out = dram.tile([128, 4], F32, tag=f"ccout{br}")
                nc.sync.dma_start(out=cc_in, in_=statp)
                if os.environ.get("KERNEL_NO_CC"):
                    nc.sync.dma_start(out=cc_out, in_=cc_in)
                else:
                    nc.gpsimd.collective_compute(
                        "AllReduce", OP.add,
                        replica_groups=[list(range(NCORES))],
                        ins=[cc_in.opt()], outs=[cc_out.opt()])
                statg[br] = cc_out

            # ---- BN finalize + residual + output ----
            # Branch a's finalize is issued BEFORE branch b's statg readback
            # so it executes during branch b's AllReduce latency instead of
            # serializing behind it.
            def finalize(br):
                abr = "a" if br == 0 else "b"
                sg = st.tile([128, 4], F32, tag=f"statg{br}")
                nc.sync.dma_start(out=sg, in_=statg[br])
                mean = st.tile([128, CT], F32, tag=f"mean{br}")
                nc.vector.tensor_scalar_mul(mean, sg[:, 0:2], 1.0 / NSTAT)
                esq = st.tile([128, CT], F32, tag=f"esq{br}")
                nc.vector.tensor_scalar_mul(esq, sg[:, 2:4], 1.0 / NSTAT)
                m2 = st.tile([128, CT], F32, tag=f"m2{br}")
                nc.vector.tensor_mul(out=m2, in0=mean, in1=mean)
                var = st.tile([128, CT], F32, tag=f"var{br}")
                nc.vector.tensor_sub(out=var, in0=esq, in1=m2)
                sd = st.tile([128, CT], F32, tag=f"sd{br}")
                nc.scalar.activation(out=sd, in_=var, func=AF.Sqrt, bias=eps_sb[:, 0:1], scale=1.0)
                rstd = st.tile([128, CT], F32, tag=f"rstd{br}")
                nc.vector.reciprocal(out=rstd, in_=sd)
                A_ = st.tile([128, CT], F32, tag=f"A{br}")
                nc.vector.tensor_mul(out=A_, in0=rstd, in1=gb[f"ga_{abr}"])
                mA = st.tile([128, CT], F32, tag=f"mA{br}")
                nc.vector.tensor_mul(out=mA, in0=mean, in1=A_)
                Bc = st.tile([128, CT], F32, tag=f"Bc{br}")
                nc.vector.tensor_sub(out=Bc, in0=gb[f"be_{abr}"], in1=mA)
                for s in range(S):
                    for ct in range(CT):
                        tmp = bn.tile([128, L], F32, tag="bnt", name="bnt")
                        nc.scalar.activation(
                            out=tmp, in_=o_tiles[(br, s, ct)],
                            func=AF.Identity,
                            scale=A_[:, ct:ct + 1], bias=Bc[:, ct:ct + 1])
                        res = bn.tile([128, L], F32, tag="bnr", name="bnr")
                        nc.vector.tensor_add(
                            out=res, in0=tmp, in1=img[s][ct][:, 1:L + 1])
                        nc.sync.dma_start(out=out_p[s, br, ct], in_=res)

            finalize(0)
            finalize(1)

    nc.compile()
    return nc


def _get_nc():
    if not _NC_CACHE:
        _NC_CACHE.append(_build_nc())
    return _NC_CACHE[0]


def _prep_shared(inp):
    f = NP_MM
    m = {}

    def cc_layout(w):
        # [256(mt*128+mi), 256(kt*128+ki), 3(t)] -> [kt, ki, 768=(t*2+mt)*128+mi]
        return np.ascontiguousarray(
            np.asarray(w).reshape(2, 128, 2, 128, 3).transpose(2, 3, 4, 0, 1).reshape(2, 128, 768)).astype(f)

    m["wq_a"] = cc_layout(inp["a_qw"])
    m["wo_a"] = cc_layout(inp["a_ow"])
    m["wk_b"] = cc_layout(inp["b_kw"])
    m["wo_b"] = cc_layout(inp["b_ow"])
    # b_vw [c, c', t] -> [kt, t, ki, c]
    m["wv_b"] = np.ascontiguousarray(
        np.asarray(inp["b_vw"]).reshape(C, 2, 128, 3).transpose(1, 3, 2, 0)).astype(f)
    m["wk_a"] = np.ascontiguousarray(np.asarray(inp["a_kw"])[:, 0, :].T).astype(f)
    m["wv_a"] = np.ascontiguousarray(np.asarray(inp["a_vw"])[:, 0, :].T).astype(f)
    m["wq_b"] = np.ascontiguousarray(np.asarray(inp["b_qw"])[:, 0, :].T).astype(f)
    for dst, src in (("qb_a", "a_qb"), ("kb_a", "a_kb"), ("ob_a", "a_ob"),
                     ("qb_b", "b_qb"), ("kb_b", "b_kb"), ("ob_b", "b_ob"),
                     ("ga_a", "a_g"), ("be_a", "a_beta"),
                     ("ga_b", "b_g"), ("be_b", "b_beta")):
        m[dst] = np.ascontiguousarray(np.asarray(inp[src]).reshape(2, 128).T).astype(np.float32)
    m["vb_a"] = np.ascontiguousarray(np.asarray(inp["a_vb"])[None, :]).astype(np.float32)
    m["vb_b"] = np.ascontiguousarray(np.asarray(inp["b_vb"])[None, :]).astype(np.float32)
    return m


def _core_maps(image, clinical, shared, ncores=NCORES):
    in_maps = []
    for core in range(ncores):
        m = dict(shared)
        sl = slice(core * S, (core + 1) * S)
        a = image[sl].reshape(S, CT, 128, L)
        pad = np.zeros((S, CT, 128, L + 2), np.float32)
        pad[..., 1:L + 1] = a
        m["img"] = pad
        m["imm"] = pad.astype(NP_MM)
        c = clinical[sl][:, 0, :]
        im2 = np.zeros((S, 3, L), np.float32)
        im2[:, 0, 1:] = c[:, :L - 1]
        im2[:, 1, :] = c
        im2[:, 2, :L - 1] = c[:, 1:]
        m["cli"] = im2.astype(NP_MM)
        in_maps.append(m)
    return in_maps


def kernel(**inputs):
    inp = {k: np.asarray(v) for k, v in inputs.items()}
    nc = _get_nc()
    shared = _prep_shared(inp)
    image = inp["image"].astype(np.float32)
    clinical = inp["clinical"].astype(np.float32)
    in_maps = _core_maps(image, clinical, shared)
    res = run_bass_kernel_spmd(nc, in_maps, core_ids=list(range(NCORES)))
    outs = np.concatenate([res.results[i]["out"] for i in range(NCORES)], axis=0)
    return np.ascontiguousarray(outs.reshape(16, 512, L))



# revision 9
# speedup vs baseline: 1.2563x; 1.1489x over previous
"""CrossSymmetricModal trn2 kernel: 2 cross-attention branches + BN + residual.

Data-parallel over batch (2 samples/core on 8 cores); BatchNorm batch stats
cross-core via a small AllReduce per branch.

Heavy matmuls run in bf16 on the TensorEngine (fp32 PSUM accumulation):
 - convs (K=3, 'same' pad) are tap-shifted matmuls accumulating in PSUM; the
   1->256 convs contract over the 3-row im2col of clinical.
 - attention is transpose-free: scoresT[m,l] = k^T q is produced directly in
   [m (partition), l] orientation, exp on ScalarE (no max subtraction: scores
   are bounded ~|1.6|), the softmax denominator is a ones-vector matmul, and
   ctx[c,l] = vT.T @ expT needs vT which each branch's v-conv emits directly
   in transposed [m, c] layout.
The residual image, all reductions and the BN math stay in fp32.
"""
import os
import sys

sys.path.insert(0, '/opt/trn_rl_repo')

import ml_dtypes
import numpy as np

from concourse import bacc, mybir, tile
from concourse.bass_utils import run_bass_kernel_spmd

S = 2            # samples per core
NCORES = 8
C = 256
CT = 2           # 128-partition channel tiles
L = 1024
LS = 2           # 512-wide l slices
EPS = 1e-5
SCALE = 1.0 / 16.0   # 1/sqrt(C)
NSTAT = 16 * L       # batchnorm reduction size (full batch x length)

F32 = mybir.dt.float32
F32R = mybir.dt.float32r
BF16 = mybir.dt.bfloat16
USE_BF16 = os.environ.get("KERNEL_DT", "bf16") == "bf16"
DT_MM = BF16 if USE_BF16 else F32R
NP_MM = ml_dtypes.bfloat16 if USE_BF16 else np.float32
AF = mybir.ActivationFunctionType
OP = mybir.AluOpType

_NC_CACHE = []


def _build_nc():
    nc = bacc.Bacc(num_devices=NCORES)

    # ---- I/O ----
    img_p = nc.declare_dram_parameter("img", [S, CT, 128, L + 2], F32, isOutput=False)
    imm_p = nc.declare_dram_parameter("imm", [S, CT, 128, L + 2], DT_MM, isOutput=False)
    cli_p = nc.declare_dram_parameter("cli", [S, 3, L], DT_MM, isOutput=False)
    w_cc_p = {}   # 256->256 conv weights, [CT(kt), 128(ki), 768=(t,mt)*128+mi]
    for name in ("wq_a", "wo_a", "wk_b", "wo_b"):
        w_cc_p[name] = nc.declare_dram_parameter(name, [CT, 128, 768], DT_MM, isOutput=False)
    wv_b_p = nc.declare_dram_parameter("wv_b", [CT, 3, 128, C], DT_MM, isOutput=False)
    w_sm_p = {}   # 1->256 convs as [3, 256]
    for name in ("wk_a", "wv_a", "wq_b"):
        w_sm_p[name] = nc.declare_dram_parameter(name, [3, C], DT_MM, isOutput=False)
    bias_p = {}
    for name in ("qb_a", "kb_a", "ob_a", "qb_b", "kb_b", "ob_b"):
        bias_p[name] = nc.declare_dram_parameter(name, [128, CT], F32, isOutput=False)
    vb_p = {}
    for name in ("vb_a", "vb_b"):
        vb_p[name] = nc.declare_dram_parameter(name, [1, C], F32R, isOutput=False)
    gb_p = {}
    for name in ("ga_a", "be_a", "ga_b", "be_b"):
        gb_p[name] = nc.declare_dram_parameter(name, [128, CT], F32, isOutput=False)
    out_p = nc.declare_dram_parameter("out", [S, 2, CT, 128, L], F32, isOutput=True)

    from contextlib import ExitStack
    with tile.TileContext(nc) as tc, ExitStack() as es:
        ec = es.enter_context
        wgt = ec(tc.tile_pool(name="wgt", bufs=1))
        io = ec(tc.tile_pool(name="io", bufs=1))
        qk = ec(tc.tile_pool(name="qk", bufs=2))
        vtp = ec(tc.tile_pool(name="vtp", bufs=2))
        ex = ec(tc.tile_pool(name="ex", bufs=3))
        cx = ec(tc.tile_pool(name="cx", bufs=2))
        op_pool = ec(tc.tile_pool(name="op", bufs=1))
        sm = ec(tc.tile_pool(name="sm", bufs=2))
        st = ec(tc.tile_pool(name="st", bufs=1))
        bn = ec(tc.tile_pool(name="bn", bufs=3))
        dram = ec(tc.tile_pool(name="dram", bufs=1, space="DRAM"))
        ps_conv = ec(tc.tile_pool(name="psc", bufs=2, space="PSUM"))
        ps_sc = ec(tc.tile_pool(name="pss", bufs=2, space="PSUM"))
        ps_ctx = ec(tc.tile_pool(name="psx", bufs=1, space="PSUM"))
        ps_den = ec(tc.tile_pool(name="psd", bufs=1, space="PSUM"))
        ps_bc = ec(tc.tile_pool(name="psb", bufs=1, space="PSUM"))
        if True:
            # ---- load weights/constants to SBUF ----
            # DMA issue order matters: the first conv (branch a, s0) needs
            # wq_a + qb_a + imm[0] only, so those go first; the residual img
            # tiles are needed only at BN-finalize and stream last.
            w_cc = {name: [] for name in ("wq_a", "wo_a", "wk_b", "wo_b")}
            for kt in range(CT):
                t_ = wgt.tile([128, 768], DT_MM, tag=f"wq_a_{kt}")
                nc.sync.dma_start(out=t_, in_=w_cc_p["wq_a"][kt])
                w_cc["wq_a"].append(t_)
            bias = {}
            for name in ("qb_a", "kb_a", "ob_a", "qb_b", "kb_b", "ob_b"):
                t_ = wgt.tile([128, CT], F32, tag=name)
                nc.sync.dma_start(out=t_, in_=bias_p[name][:, :])
                bias[name] = t_
            imm = []   # [s][kt] -> [128, L+2] matmul dtype
            cli = []   # [s] -> [3, L]
            for s in range(S):
                imm.append([])
                for kt in range(CT):
                    t_ = io.tile([128, L + 2], DT_MM, tag=f"imm_{s}_{kt}")
                    nc.sync.dma_start(out=t_, in_=imm_p[s, kt])
                    imm[s].append(t_)
                t_ = io.tile([3, L], DT_MM, tag=f"cli_{s}")
                nc.sync.dma_start(out=t_, in_=cli_p[s])
                cli.append(t_)
            w_sm = {}
            for name in ("wk_a", "wv_a", "wq_b"):
                t_ = wgt.tile([3, C], DT_MM, tag=name)
                nc.sync.dma_start(out=t_, in_=w_sm_p[name][:, :])
                w_sm[name] = t_
            vb = {}
            for name in ("vb_a", "vb_b"):
                t_ = wgt.tile([1, C], F32R, tag=name)
                nc.sync.dma_start(out=t_, in_=vb_p[name][:, :])
                vb[name] = t_
            for name in ("wo_a", "wk_b", "wo_b"):
                for kt in range(CT):
                    t_ = wgt.tile([128, 768], DT_MM, tag=f"{name}_{kt}")
                    nc.sync.dma_start(out=t_, in_=w_cc_p[name][kt])
                    w_cc[name].append(t_)
            wv_b = []
            for kt in range(CT):
                row = []
                for t in range(3):
                    t_ = wgt.tile([128, C], DT_MM, tag=f"wv_b_{kt}_{t}")
                    nc.sync.dma_start(out=t_, in_=wv_b_p[kt, t])
                    row.append(t_)
                wv_b.append(row)
            gb = {}
            for name in ("ga_a", "be_a", "ga_b", "be_b"):
                t_ = wgt.tile([128, CT], F32, tag=name)
                nc.sync.dma_start(out=t_, in_=gb_p[name][:, :])
                gb[name] = t_
            img = []   # [s][kt] -> [128, L+2] f32 (residual)
            for s in range(S):
                img.append([])
                for kt in range(CT):
                    t_ = io.tile([128, L + 2], F32, tag=f"img_{s}_{kt}")
                    nc.sync.dma_start(out=t_, in_=img_p[s, kt])
                    img[s].append(t_)
            ones_f32c = wgt.tile([128, 1], F32, tag="ones_f32c")
            nc.vector.memset(ones_f32c, 1.0)
            ones_col = wgt.tile([128, 1], DT_MM, tag="ones_col")
            nc.vector.tensor_copy(out=ones_col, in_=ones_f32c)
            ones_f32r_ = wgt.tile([1, 128], F32, tag="ones_f32r_")
            nc.vector.memset(ones_f32r_, 1.0)
            ones_row = wgt.tile([1, 128], F32R, tag="ones_row")
            nc.vector.tensor_copy(out=ones_row, in_=ones_f32r_)
            eps_sb = wgt.tile([128, 1], F32, tag="eps_sb")
            nc.vector.memset(eps_sb, EPS)
            zero_col = wgt.tile([128, 1], F32, tag="zero_col")
            nc.vector.memset(zero_col, 0.0)

            # ---- helpers ----
            def conv_cc(dst, w_kt, bias_ap, src, stats=None):
                """256->256 K=3 conv: dst[ct][:, l] from padded src[kt] tiles."""
                for ct in range(CT):
                    for ls in range(LS):
                        p = ps_conv.tile([128, 512], F32, tag="conv", name="convp")
                        n = 0
                        for kt in range(CT):
                            for t in range(3):
                                nc.tensor.matmul(
                                    p,
                                    lhsT=w_kt[kt][:, (t * 2 + ct) * 128:(t * 2 + ct + 1) * 128],
                                    rhs=src[kt][:, ls * 512 + t: ls * 512 + t + 512],
                                    start=(n == 0), stop=(n == 5))
                                n += 1
                        acc = None if stats is None else stats(ct, ls)
                        nc.scalar.activation(
                            out=dst[ct][:, ls * 512:(ls + 1) * 512], in_=p,
                            func=AF.Identity, bias=bias_ap[:, ct:ct + 1], scale=1.0,
                            accum_out=acc)

            def conv_1c(dst, w_lhsT, bias_ap, cli_t):
                """1->256 K=3 conv via [3,*] im2col rows."""
                for ct in range(CT):
                    for ls in range(LS):
                        p = ps_conv.tile([128, 512], F32, tag="conv", name="convp")
                        nc.tensor.matmul(
                            p, lhsT=w_lhsT[:, ct * 128:(ct + 1) * 128],
                            rhs=cli_t[:, ls * 512:(ls + 1) * 512],
                            start=True, stop=True)
                        nc.scalar.activation(
                            out=dst[ct][:, ls * 512:(ls + 1) * 512], in_=p,
                            func=AF.Identity, bias=bias_ap[:, ct:ct + 1], scale=1.0)

            o_tiles = {}
            slots = {}
            statg = {}

            for br in range(2):
                abr = "a" if br == 0 else "b"
                slots[br] = st.tile([128, 4 * S * LS], F32, tag=f"slots{br}", name=f"slots{br}")
                for s in range(S):
                    # ---- convs ----
                    q_sb = [qk.tile([128, L], DT_MM, tag=f"q{kt}", name=f"q{kt}") for kt in range(CT)]
                    k_sb = [qk.tile([128, L], DT_MM, tag=f"k{kt}", name=f"k{kt}") for kt in range(CT)]
                    vt = [vtp.tile([128, C], DT_MM, tag=f"vt{mt}", name=f"vt{mt}") for mt in range(8)]
                    if br == 0:
                        conv_cc(q_sb, w_cc["wq_a"], bias["qb_a"], imm[s])
                        conv_1c(k_sb, w_sm["wk_a"], bias["kb_a"], cli[s])
                        for mt in range(8):
                            p = ps_conv.tile([128, C], F32, tag="conv", name="convp")
                            nc.tensor.matmul(
                                p, lhsT=cli[s][:, mt * 128:(mt + 1) * 128],
                                rhs=w_sm["wv_a"], start=True, stop=True)
                            nc.vector.tensor_copy(out=vt[mt], in_=p)
                    else:
                        conv_1c(q_sb, w_sm["wq_b"], bias["qb_b"], cli[s])
                        conv_cc(k_sb, w_cc["wk_b"], bias["kb_b"], imm[s])
                        for mt in range(8):
                            p = ps_conv.tile([128, C], F32, tag="conv", name="convp")
                            n = 0
                            for kt in range(CT):
                                for t in range(3):
                                    nc.tensor.matmul(
                                        p,
                                        lhsT=imm[s][kt][:, mt * 128 + t: mt * 128 + t + 128],
                                        rhs=wv_b[kt][t],
                                        start=(n == 0), stop=(n == 5))
                                    n += 1
                            nc.vector.tensor_copy(out=vt[mt], in_=p)

                    # ---- attention (m-loop software-pipelined: scores for
                    # step mt+1 issue on PE before ctx of step mt, so the PE
                    # stays busy while ScalarE computes exp of step mt) ----
                    ctx = [cx.tile([128, L + 2], DT_MM, tag=f"ctx{kt}", name=f"ctx{kt}") for kt in range(CT)]
                    for ct in range(CT):
                        nc.vector.tensor_copy(out=ctx[ct][:, 0:1], in_=zero_col)
                        nc.vector.tensor_copy(out=ctx[ct][:, L + 1:L + 2], in_=zero_col)
                    for ls in range(LS):
                        ctx_ps = [ps_ctx.tile([128, 512], F32, tag=f"ctxp{ct}", name=f"ctxp{ct}") for ct in range(CT)]
                        den_ps = ps_den.tile([1, 512], F32, tag="den", name="den_ps")
                        ets = {}

                        def _sc_exp(mt):
                            sc = ps_sc.tile([128, 512], F32, tag="sc", name="sc")
                            for kt in range(CT):
                                nc.tensor.matmul(
                                    sc, lhsT=k_sb[kt][:, mt * 128:(mt + 1) * 128],
                                    rhs=q_sb[kt][:, ls * 512:(ls + 1) * 512],
                                    start=(kt == 0), stop=(kt == CT - 1))
                            et = ex.tile([128, 512], DT_MM, tag="expT", name="et")
                            nc.scalar.activation(out=et, in_=sc, func=AF.Exp, scale=SCALE)
                            ets[mt] = et

                        _sc_exp(0)
                        _sc_exp(1)
                        for mt in range(8):
                            if mt + 2 < 8:
                                _sc_exp(mt + 2)
                            et = ets.pop(mt)
                            for ct in range(CT):
                                nc.tensor.matmul(
                                    ctx_ps[ct], lhsT=vt[mt][:, ct * 128:(ct + 1) * 128],
                                    rhs=et, start=(mt == 0), stop=False)
                            nc.tensor.matmul(
                                den_ps, lhsT=ones_col, rhs=et,
                                start=(mt == 0), stop=(mt == 7))
                        den_sb = sm.tile([1, 512], F32R, tag="den_sb")
                        nc.vector.tensor_copy(out=den_sb, in_=den_ps)
                        for ct in range(CT):
                            nc.tensor.matmul(
                                ctx_ps[ct], lhsT=vb[f"vb_{abr}"][:, ct * 128:(ct + 1) * 128],
                                rhs=den_sb, start=False, stop=True)
                        # 1/den on the single row (den ~1e2..1e3, no edge
                        # cases), then broadcast via PE; multiply from PSUM
                        den_f = sm.tile([1, 512], F32, tag="den_f")
                        nc.vector.tensor_copy(out=den_f, in_=den_ps)
                        rden = sm.tile([1, 512], F32, tag="rden")
                        nc.vector.reciprocal_approx_fast(out=rden, in_=den_f)
                        rden_r = sm.tile([1, 512], F32R, tag="rden_r")
                        nc.vector.tensor_copy(out=rden_r, in_=rden)
                        bc_ps = ps_bc.tile([128, 512], F32, tag="bc", name="bc_ps")
                        nc.tensor.matmul(bc_ps, lhsT=ones_row, rhs=rden_r, start=True, stop=True)
                        bc_sb = sm.tile([128, 512], F32, tag="bc_sb")
                        nc.vector.tensor_copy(out=bc_sb, in_=bc_ps)
                        for ct in range(CT):
                            nc.vector.tensor_mul(
                                out=ctx[ct][:, 1 + ls * 512: 1 + (ls + 1) * 512],
                                in0=ctx_ps[ct], in1=bc_sb)

                    # ---- out conv + stats ----
                    o_sb = [op_pool.tile([128, L], F32, tag=f"o_{br}_{s}_{ct}", name=f"o_{br}_{s}_{ct}") for ct in range(CT)]
                    for ct in range(CT):
                        o_tiles[(br, s, ct)] = o_sb[ct]

                    def _acc(ct, ls, _br=br, _s=s):
                        i = ct * S * LS + _s * LS + ls
                        return slots[_br][:, i:i + 1]

                    conv_cc(o_sb, w_cc[f"wo_{abr}"], bias[f"ob_{abr}"], ctx, stats=_acc)
                    for ct in range(CT):
                        for ls in range(LS):
                            sq = sm.tile([128, 512], F32, tag="sqscr", name="sq")
                            osl = o_sb[ct][:, ls * 512:(ls + 1) * 512]
                            nc.vector.tensor_mul(out=sq, in0=osl, in1=osl)
                            i = (2 + ct) * S * LS + s * LS + ls
                            nc.vector.reduce_sum(
                                out=slots[br][:, i:i + 1], in_=sq,
                                axis=mybir.AxisListType.X)

                # ---- cross-core stats all-reduce for this branch ----
                statp = st.tile([128, 4], F32, tag=f"statp{br}")
                nc.vector.reduce_sum(out=statp, in_=slots[br].rearrange("p (g i) -> p g i", i=S * LS), axis=mybir.AxisListType.X)
                cc_in = dram.tile([128, 4], F32, tag=f"ccin{br}")
                cc_out = dram.tile([128, 4], F32, tag=f"ccout{br}")
                nc.sync.dma_start(out=cc_in, in_=statp)
                if os.environ.get("KERNEL_NO_CC"):
                    nc.sync.dma_start(out=cc_out, in_=cc_in)
                else:
                    nc.gpsimd.collective_compute(
                        "AllReduce", OP.add,
                        replica_groups=[list(range(NCORES))],
                        ins=[cc_in.opt()], outs=[cc_out.opt()])
                if br == 0:
                    # readback issued HERE (before branch b's collective is
                    # in program order) so its semaphore wait covers only
                    # this branch's AllReduce.
                    sg = st.tile([128, 4], F32, tag=f"statg{br}")
                    nc.sync.dma_start(out=sg, in_=cc_out)
                    statg[br] = sg
                else:
                    statg[br] = cc_out

            # ---- BN finalize + residual + output ----
            # Branch a's finalize is issued BEFORE branch b's statg readback
            # so it executes during branch b's AllReduce latency instead of
            # serializing behind it.
            def finalize(br):
                abr = "a" if br == 0 else "b"
                if br == 0:
                    sg = statg[br]
                else:
                    sg = st.tile([128, 4], F32, tag=f"statg{br}")
                    nc.sync.dma_start(out=sg, in_=statg[br])
                mean = st.tile([128, CT], F32, tag=f"mean{br}")
                nc.vector.tensor_scalar_mul(mean, sg[:, 0:2], 1.0 / NSTAT)
                esq = st.tile([128, CT], F32, tag=f"esq{br}")
                nc.vector.tensor_scalar_mul(esq, sg[:, 2:4], 1.0 / NSTAT)
                m2 = st.tile([128, CT], F32, tag=f"m2{br}")
                nc.vector.tensor_mul(out=m2, in0=mean, in1=mean)
                var = st.tile([128, CT], F32, tag=f"var{br}")
                nc.vector.tensor_sub(out=var, in0=esq, in1=m2)
                sd = st.tile([128, CT], F32, tag=f"sd{br}")
                nc.scalar.activation(out=sd, in_=var, func=AF.Sqrt, bias=eps_sb[:, 0:1], scale=1.0)
                rstd = st.tile([128, CT], F32, tag=f"rstd{br}")
                nc.vector.reciprocal(out=rstd, in_=sd)
                A_ = st.tile([128, CT], F32, tag=f"A{br}")
                nc.vector.tensor_mul(out=A_, in0=rstd, in1=gb[f"ga_{abr}"])
                mA = st.tile([128, CT], F32, tag=f"mA{br}")
                nc.vector.tensor_mul(out=mA, in0=mean, in1=A_)
                Bc = st.tile([128, CT], F32, tag=f"Bc{br}")
                nc.vector.tensor_sub(out=Bc, in0=gb[f"be_{abr}"], in1=mA)
                for s in range(S):
                    for ct in range(CT):
                        tmp = bn.tile([128, L], F32, tag="bnt", name="bnt")
                        nc.scalar.activation(
                            out=tmp, in_=o_tiles[(br, s, ct)],
                            func=AF.Identity,
                            scale=A_[:, ct:ct + 1], bias=Bc[:, ct:ct + 1])
                        res = bn.tile([128, L], F32, tag="bnr", name="bnr")
                        nc.vector.tensor_add(
                            out=res, in0=tmp, in1=img[s][ct][:, 1:L + 1])
                        nc.sync.dma_start(out=out_p[s, br, ct], in_=res)

            finalize(0)
            finalize(1)

    nc.compile()
    return nc


def _get_nc():
    if not _NC_CACHE:
        _NC_CACHE.append(_build_nc())
    return _NC_CACHE[0]


def _prep_shared(inp):
    f = NP_MM
    m = {}

    def cc_layout(w):
        # [256(mt*128+mi), 256(kt*128+ki), 3(t)] -> [kt, ki, 768=(t*2+mt)*128+mi]
        return np.ascontiguousarray(
            np.asarray(w).reshape(2, 128, 2, 128, 3).transpose(2, 3, 4, 0, 1).reshape(2, 128, 768)).astype(f)

    m["wq_a"] = cc_layout(inp["a_qw"])
    m["wo_a"] = cc_layout(inp["a_ow"])
    m["wk_b"] = cc_layout(inp["b_kw"])
    m["wo_b"] = cc_layout(inp["b_ow"])
    # b_vw [c, c', t] -> [kt, t, ki, c]
    m["wv_b"] = np.ascontiguousarray(
        np.asarray(inp["b_vw"]).reshape(C, 2, 128, 3).transpose(1, 3, 2, 0)).astype(f)
    m["wk_a"] = np.ascontiguousarray(np.asarray(inp["a_kw"])[:, 0, :].T).astype(f)
    m["wv_a"] = np.ascontiguousarray(np.asarray(inp["a_vw"])[:, 0, :].T).astype(f)
    m["wq_b"] = np.ascontiguousarray(np.asarray(inp["b_qw"])[:, 0, :].T).astype(f)
    for dst, src in (("qb_a", "a_qb"), ("kb_a", "a_kb"), ("ob_a", "a_ob"),
                     ("qb_b", "b_qb"), ("kb_b", "b_kb"), ("ob_b", "b_ob"),
                     ("ga_a", "a_g"), ("be_a", "a_beta"),
                     ("ga_b", "b_g"), ("be_b", "b_beta")):
        m[dst] = np.ascontiguousarray(np.asarray(inp[src]).reshape(2, 128).T).astype(np.float32)
    m["vb_a"] = np.ascontiguousarray(np.asarray(inp["a_vb"])[None, :]).astype(np.float32)
    m["vb_b"] = np.ascontiguousarray(np.asarray(inp["b_vb"])[None, :]).astype(np.float32)
    return m


def _core_maps(image, clinical, shared, ncores=NCORES):
    in_maps = []
    for core in range(ncores):
        m = dict(shared)
        sl = slice(core * S, (core + 1) * S)
        a = image[sl].reshape(S, CT, 128, L)
        pad = np.zeros((S, CT, 128, L + 2), np.float32)
        pad[..., 1:L + 1] = a
        m["img"] = pad
        m["imm"] = pad.astype(NP_MM)
        c = clinical[sl][:, 0, :]
        im2 = np.zeros((S, 3, L), np.float32)
        im2[:, 0, 1:] = c[:, :L - 1]
        im2[:, 1, :] = c
        im2[:, 2, :L - 1] = c[:, 1:]
        m["cli"] = im2.astype(NP_MM)
        in_maps.append(m)
    return in_maps


def kernel(**inputs):
    inp = {k: np.asarray(v) for k, v in inputs.items()}
    nc = _get_nc()
    shared = _prep_shared(inp)
    image = inp["image"].astype(np.float32)
    clinical = inp["clinical"].astype(np.float32)
    in_maps = _core_maps(image, clinical, shared)
    res = run_bass_kernel_spmd(nc, in_maps, core_ids=list(range(NCORES)))
    outs = np.concatenate([res.results[i]["out"] for i in range(NCORES)], axis=0)
    return np.ascontiguousarray(outs.reshape(16, 512, L))



# revision 10
# speedup vs baseline: 1.3028x; 1.0369x over previous
"""CrossSymmetricModal trn2 kernel: 2 cross-attention branches + BN + residual.

Data-parallel over batch (2 samples/core on 8 cores); BatchNorm batch stats
cross-core via a small AllReduce per branch.

Rank-4 factorization: every operand derived from the 1-channel clinical
stream is rank <= 4 (3 conv taps + bias/ones row), so
 - branch a (q from image, k/v from clinical):
     scoresT[m,l] = cli_aug[:,m]^T G[:,l]      with G = Wk_kb_aug^T q  [4,L]
     H[r,l]       = sum_m cliT_aug[m,r] e[m,l]  (row 3 of cliT_aug is 1s,
                                                 so H[3,:] is the softmax den)
     ctx_unnorm   = WvVb_aug^T H
 - branch b (q from clinical, k/v from image):
     scoresT[m,l] = G_b[:,m]^T cli_aug[:,l]    with G_b = Wq_qb_aug^T k
   (k/v stay full-rank image convs; den via ones-vector matmul.)
This replaces the per-128-block score matmul chains and branch-a's whole
ctx contraction with [4,*] PSUM tiles.

Heavy matmuls in bf16 (fp32 PSUM); residual image, reductions, BN in fp32.
"""
import os
import sys

sys.path.insert(0, '/opt/trn_rl_repo')

import ml_dtypes
import numpy as np

from concourse import bacc, mybir, tile
from concourse.bass_utils import run_bass_kernel_spmd

S = 2            # samples per core
NCORES = 8
C = 256
CT = 2           # 128-partition channel tiles
L = 1024
LS = 2           # 512-wide l slices
EPS = 1e-5
SCALE = 1.0 / 16.0   # 1/sqrt(C)
NSTAT = 16 * L       # batchnorm reduction size (full batch x length)

F32 = mybir.dt.float32
F32R = mybir.dt.float32r
BF16 = mybir.dt.bfloat16
USE_BF16 = os.environ.get("KERNEL_DT", "bf16") == "bf16"
DT_MM = BF16 if USE_BF16 else F32R
NP_MM = ml_dtypes.bfloat16 if USE_BF16 else np.float32
AF = mybir.ActivationFunctionType
OP = mybir.AluOpType

_NC_CACHE = []


def _build_nc():
    nc = bacc.Bacc(num_devices=NCORES)

    # ---- I/O ----
    img_p = nc.declare_dram_parameter("img", [S, CT, 128, L + 2], F32, isOutput=False)
    imm_p = nc.declare_dram_parameter("imm", [S, CT, 128, L + 2], DT_MM, isOutput=False)
    cli4_p = nc.declare_dram_parameter("cli4", [S, 4, L], DT_MM, isOutput=False)
    clit4_p = nc.declare_dram_parameter("clit4", [S, 128, 32], DT_MM, isOutput=False)
    w_cc_p = {}   # 256->256 conv weights, [CT(kt), 128(ki), 768=(t,mt)*128+mi]
    for name in ("wq_a", "wo_a", "wk_b", "wo_b"):
        w_cc_p[name] = nc.declare_dram_parameter(name, [CT, 128, 768], DT_MM, isOutput=False)
    wv_b_p = nc.declare_dram_parameter("wv_b", [CT, 3, 128, C], DT_MM, isOutput=False)
    wk4_a_p = nc.declare_dram_parameter("wk4_a", [CT, 128, 4], DT_MM, isOutput=False)
    wq4_b_p = nc.declare_dram_parameter("wq4_b", [CT, 128, 4], DT_MM, isOutput=False)
    wv4_a_p = nc.declare_dram_parameter("wv4_a", [4, C], DT_MM, isOutput=False)
    bias_p = {}
    for name in ("qb_a", "ob_a", "kb_b", "ob_b"):
        bias_p[name] = nc.declare_dram_parameter(name, [128, CT], F32, isOutput=False)
    vb_b_p = nc.declare_dram_parameter("vb_b", [1, C], F32R, isOutput=False)
    gb_p = {}
    for name in ("ga_a", "be_a", "ga_b", "be_b"):
        gb_p[name] = nc.declare_dram_parameter(name, [128, CT], F32, isOutput=False)
    out_p = nc.declare_dram_parameter("out", [S, 2, CT, 128, L], F32, isOutput=True)

    from contextlib import ExitStack
    with tile.TileContext(nc) as tc, ExitStack() as es:
        ec = es.enter_context
        wgt = ec(tc.tile_pool(name="wgt", bufs=1))
        io = ec(tc.tile_pool(name="io", bufs=1))
        qk = ec(tc.tile_pool(name="qk", bufs=2))
        vtp = ec(tc.tile_pool(name="vtp", bufs=2))
        ex = ec(tc.tile_pool(name="ex", bufs=3))
        cx = ec(tc.tile_pool(name="cx", bufs=2))
        op_pool = ec(tc.tile_pool(name="op", bufs=1))
        sm = ec(tc.tile_pool(name="sm", bufs=2))
        g4p = ec(tc.tile_pool(name="g4p", bufs=2))
        st = ec(tc.tile_pool(name="st", bufs=1))
        bn = ec(tc.tile_pool(name="bn", bufs=3))
        dram = ec(tc.tile_pool(name="dram", bufs=1, space="DRAM"))
        ps_conv = ec(tc.tile_pool(name="psc", bufs=2, space="PSUM"))
        ps_sc = ec(tc.tile_pool(name="pss", bufs=2, space="PSUM"))
        ps_ctx = ec(tc.tile_pool(name="psx", bufs=1, space="PSUM"))
        ps_gh = ec(tc.tile_pool(name="psg", bufs=1, space="PSUM"))
        ps_bc = ec(tc.tile_pool(name="psb", bufs=1, space="PSUM"))
        if True:
            # ---- load weights/constants to SBUF ----
            # DMA issue order: the first conv (branch a, s0) needs wq_a +
            # qb_a + imm[0] only; residual img streams last.
            w_cc = {name: [] for name in ("wq_a", "wo_a", "wk_b", "wo_b")}
            for kt in range(CT):
                t_ = wgt.tile([128, 768], DT_MM, tag=f"wq_a_{kt}")
                nc.sync.dma_start(out=t_, in_=w_cc_p["wq_a"][kt])
                w_cc["wq_a"].append(t_)
            imm = []   # [s][kt] -> [128, L+2] matmul dtype
            for s in range(S):
                imm.append([])
                for kt in range(CT):
                    t_ = io.tile([128, L + 2], DT_MM, tag=f"imm_{s}_{kt}")
                    nc.sync.dma_start(out=t_, in_=imm_p[s, kt])
                    imm[s].append(t_)
            bias = {}
            for name in ("qb_a",):
                t_ = wgt.tile([128, CT], F32, tag=name)
                nc.sync.dma_start(out=t_, in_=bias_p[name][:, :])
                bias[name] = t_
            wk4_a = []
            wq4_b = []
            for kt in range(CT):
                t_ = wgt.tile([128, 4], DT_MM, tag=f"wk4_a_{kt}")
                nc.sync.dma_start(out=t_, in_=wk4_a_p[kt])
                wk4_a.append(t_)
            cli4 = []  # [s] -> [4, L]
            clit4 = []  # [s] -> [128, 32] (mt-major 4-col groups)
            for s in range(S):
                t_ = io.tile([4, L], DT_MM, tag=f"cli4_{s}")
                nc.sync.dma_start(out=t_, in_=cli4_p[s])
                cli4.append(t_)
                t_ = io.tile([128, 32], DT_MM, tag=f"clit4_{s}")
                nc.sync.dma_start(out=t_, in_=clit4_p[s])
                clit4.append(t_)
            for name in ("ob_a", "kb_b", "ob_b"):
                t_ = wgt.tile([128, CT], F32, tag=name)
                nc.sync.dma_start(out=t_, in_=bias_p[name][:, :])
                bias[name] = t_
            for kt in range(CT):
                t_ = wgt.tile([128, 4], DT_MM, tag=f"wq4_b_{kt}")
                nc.sync.dma_start(out=t_, in_=wq4_b_p[kt])
                wq4_b.append(t_)
            wv4_a = wgt.tile([4, C], DT_MM, tag="wv4_a")
            nc.sync.dma_start(out=wv4_a, in_=wv4_a_p[:, :])
            vb_b = wgt.tile([1, C], F32R, tag="vb_b")
            nc.sync.dma_start(out=vb_b, in_=vb_b_p[:, :])
            for name in ("wo_a", "wk_b", "wo_b"):
                for kt in range(CT):
                    t_ = wgt.tile([128, 768], DT_MM, tag=f"{name}_{kt}")
                    nc.sync.dma_start(out=t_, in_=w_cc_p[name][kt])
                    w_cc[name].append(t_)
            wv_b = []
            for kt in range(CT):
                row = []
                for t in range(3):
                    t_ = wgt.tile([128, C], DT_MM, tag=f"wv_b_{kt}_{t}")
                    nc.sync.dma_start(out=t_, in_=wv_b_p[kt, t])
                    row.append(t_)
                wv_b.append(row)
            gb = {}
            for name in ("ga_a", "be_a", "ga_b", "be_b"):
                t_ = wgt.tile([128, CT], F32, tag=name)
                nc.sync.dma_start(out=t_, in_=gb_p[name][:, :])
                gb[name] = t_
            img = []   # [s][kt] -> [128, L+2] f32 (residual)
            for s in range(S):
                img.append([])
                for kt in range(CT):
                    t_ = io.tile([128, L + 2], F32, tag=f"img_{s}_{kt}")
                    nc.sync.dma_start(out=t_, in_=img_p[s, kt])
                    img[s].append(t_)
            ones_f32c = wgt.tile([128, 1], F32, tag="ones_f32c")
            nc.vector.memset(ones_f32c, 1.0)
            ones_col = wgt.tile([128, 1], DT_MM, tag="ones_col")
            nc.vector.tensor_copy(out=ones_col, in_=ones_f32c)
            ones_f32r_ = wgt.tile([1, 128], F32, tag="ones_f32r_")
            nc.vector.memset(ones_f32r_, 1.0)
            ones_row = wgt.tile([1, 128], F32R, tag="ones_row")
            nc.vector.tensor_copy(out=ones_row, in_=ones_f32r_)
            eps_sb = wgt.tile([128, 1], F32, tag="eps_sb")
            nc.vector.memset(eps_sb, EPS)
            zero_col = wgt.tile([128, 1], F32, tag="zero_col")
            nc.vector.memset(zero_col, 0.0)

            # ---- helpers ----
            def conv_cc(dst, w_kt, bias_ap, src, stats=None):
                """256->256 K=3 conv: dst[ct][:, l] from padded src[kt] tiles."""
                for ct in range(CT):
                    for ls in range(LS):
                        p = ps_conv.tile([128, 512], F32, tag="conv", name="convp")
                        n = 0
                        for kt in range(CT):
                            for t in range(3):
                                nc.tensor.matmul(
                                    p,
                                    lhsT=w_kt[kt][:, (t * 2 + ct) * 128:(t * 2 + ct + 1) * 128],
                                    rhs=src[kt][:, ls * 512 + t: ls * 512 + t + 512],
                                    start=(n == 0), stop=(n == 5))
                                n += 1
                        acc = None if stats is None else stats(ct, ls)
                        nc.scalar.activation(
                            out=dst[ct][:, ls * 512:(ls + 1) * 512], in_=p,
                            func=AF.Identity, bias=bias_ap[:, ct:ct + 1], scale=1.0,
                            accum_out=acc)

            def rank4_G(q_sb, w4, tag):
                """G[r, l] = sum_c w4[c, r] q[c, l]  -> [4, L] bf16 in SBUF."""
                g_sb = g4p.tile([4, L], DT_MM, tag=tag, name=tag)
                for ls in range(LS):
                    g_ps = ps_gh.tile([4, 512], F32, tag="gh", name="gh")
                    for kt in range(CT):
                        nc.tensor.matmul(
                            g_ps, lhsT=w4[kt],
                            rhs=q_sb[kt][:, ls * 512:(ls + 1) * 512],
                            start=(kt == 0), stop=(kt == CT - 1))
                    nc.vector.tensor_copy(
                        out=g_sb[:, ls * 512:(ls + 1) * 512], in_=g_ps)
                return g_sb

            o_tiles = {}
            slots = {}
            statg = {}

            for br in range(2):
                abr = "a" if br == 0 else "b"
                slots[br] = st.tile([128, 4 * S * LS], F32, tag=f"slots{br}", name=f"slots{br}")
                # ---- convs + rank-4 G for BOTH samples first: gives the PE
                # independent conv work to fill attention dependency stalls
                gs = []
                vts = []
                for s in range(S):
                    if br == 0:
                        q_sb = [qk.tile([128, L], DT_MM, tag=f"q{kt}", name=f"q{kt}") for kt in range(CT)]
                        conv_cc(q_sb, w_cc["wq_a"], bias["qb_a"], imm[s])
                        gs.append(rank4_G(q_sb, wk4_a, "ga4"))
                        vts.append(None)
                    else:
                        k_sb = [qk.tile([128, L], DT_MM, tag=f"k{kt}", name=f"k{kt}") for kt in range(CT)]
                        conv_cc(k_sb, w_cc["wk_b"], bias["kb_b"], imm[s])
                        gs.append(rank4_G(k_sb, wq4_b, "gb4"))
                        vt = [vtp.tile([128, C], DT_MM, tag=f"vt{mt}", name=f"vt{mt}") for mt in range(8)]
                        for mt in range(8):
                            p = ps_conv.tile([128, C], F32, tag="conv", name="convp")
                            n = 0
                            for kt in range(CT):
                                for t in range(3):
                                    nc.tensor.matmul(
                                        p,
                                        lhsT=imm[s][kt][:, mt * 128 + t: mt * 128 + t + 128],
                                        rhs=wv_b[kt][t],
                                        start=(n == 0), stop=(n == 5))
                                    n += 1
                            nc.vector.tensor_copy(out=vt[mt], in_=p)
                        vts.append(vt)

                for s in range(S):
                    g_sb = gs[s]
                    vt = vts[s]
                    # ---- attention (m-loop software-pipelined) ----
                    ctx = [cx.tile([128, L + 2], DT_MM, tag=f"ctx{kt}", name=f"ctx{kt}") for kt in range(CT)]
                    for ct in range(CT):
                        nc.vector.tensor_copy(out=ctx[ct][:, 0:1], in_=zero_col)
                        nc.vector.tensor_copy(out=ctx[ct][:, L + 1:L + 2], in_=zero_col)
                    for ls in range(LS):
                        ets = {}

                        def _sc_exp(mt):
                            sc = ps_sc.tile([128, 512], F32, tag="sc", name="sc")
                            if br == 0:
                                nc.tensor.matmul(
                                    sc, lhsT=cli4[s][:, mt * 128:(mt + 1) * 128],
                                    rhs=g_sb[:, ls * 512:(ls + 1) * 512],
                                    start=True, stop=True)
                            else:
                                nc.tensor.matmul(
                                    sc, lhsT=g_sb[:, mt * 128:(mt + 1) * 128],
                                    rhs=cli4[s][:, ls * 512:(ls + 1) * 512],
                                    start=True, stop=True)
                            et = ex.tile([128, 512], DT_MM, tag="expT", name="et")
                            nc.scalar.activation(out=et, in_=sc, func=AF.Exp, scale=SCALE)
                            ets[mt] = et

                        _sc_exp(0)
                        _sc_exp(1)
                        if br == 0:
                            # H accumulation: [4, 512], row 3 = denominator
                            h_ps = ps_gh.tile([4, 512], F32, tag="gh", name="gh")
                            ctx_ps = [ps_ctx.tile([128, 512], F32, tag=f"ctxp{ct}", name=f"ctxp{ct}") for ct in range(CT)]
                            for mt in range(8):
                                if mt + 2 < 8:
                                    _sc_exp(mt + 2)
                                et = ets.pop(mt)
                                nc.tensor.matmul(
                                    h_ps, lhsT=clit4[s][:, mt * 4:(mt + 1) * 4],
                                    rhs=et, start=(mt == 0), stop=(mt == 7))
                            h_sb = sm.tile([4, 512], DT_MM, tag="h_sb")
                            nc.vector.tensor_copy(out=h_sb, in_=h_ps)
                            den_f = sm.tile([1, 512], F32, tag="den_f")
                            nc.vector.tensor_copy(out=den_f, in_=h_ps[0:1, :])
                            rden = sm.tile([1, 512], F32, tag="rden")
                            nc.vector.reciprocal_approx_fast(out=rden, in_=den_f)
                            rden_r = sm.tile([1, 512], F32R, tag="rden_r")
                            nc.vector.tensor_copy(out=rden_r, in_=rden)
                            bc_ps = ps_bc.tile([128, 512], F32, tag="bc", name="bc_ps")
                            nc.tensor.matmul(bc_ps, lhsT=ones_row, rhs=rden_r, start=True, stop=True)
                            bc_sb = sm.tile([128, 512], F32, tag="bc_sb")
                            nc.vector.tensor_copy(out=bc_sb, in_=bc_ps)
                            for ct in range(CT):
                                nc.tensor.matmul(
                                    ctx_ps[ct], lhsT=wv4_a[:, ct * 128:(ct + 1) * 128],
                                    rhs=h_sb, start=True, stop=True)
                                nc.vector.tensor_mul(
                                    out=ctx[ct][:, 1 + ls * 512: 1 + (ls + 1) * 512],
                                    in0=ctx_ps[ct], in1=bc_sb)
                        else:
                            ctx_ps = [ps_ctx.tile([128, 512], F32, tag=f"ctxp{ct}", name=f"ctxp{ct}") for ct in range(CT)]
                            den_t = ps_gh.tile([4, 512], F32, tag="gh", name="gh")
                            den_ps = den_t[0:1, :]
                            for mt in range(8):
                                if mt + 2 < 8:
                                    _sc_exp(mt + 2)
                                et = ets.pop(mt)
                                for ct in range(CT):
                                    nc.tensor.matmul(
                                        ctx_ps[ct], lhsT=vt[mt][:, ct * 128:(ct + 1) * 128],
                                        rhs=et, start=(mt == 0), stop=False)
                                nc.tensor.matmul(
                                    den_ps, lhsT=ones_col, rhs=et,
                                    start=(mt == 0), stop=(mt == 7))
                            den_sb = sm.tile([1, 512], F32R, tag="den_sb")
                            nc.vector.tensor_copy(out=den_sb, in_=den_ps)
                            for ct in range(CT):
                                nc.tensor.matmul(
                                    ctx_ps[ct], lhsT=vb_b[:, ct * 128:(ct + 1) * 128],
                                    rhs=den_sb, start=False, stop=True)
                            den_f = sm.tile([1, 512], F32, tag="den_f")
                            nc.vector.tensor_copy(out=den_f, in_=den_ps)
                            rden = sm.tile([1, 512], F32, tag="rden")
                            nc.vector.reciprocal_approx_fast(out=rden, in_=den_f)
                            rden_r = sm.tile([1, 512], F32R, tag="rden_r")
                            nc.vector.tensor_copy(out=rden_r, in_=rden)
                            bc_ps = ps_bc.tile([128, 512], F32, tag="bc", name="bc_ps")
                            nc.tensor.matmul(bc_ps, lhsT=ones_row, rhs=rden_r, start=True, stop=True)
                            bc_sb = sm.tile([128, 512], F32, tag="bc_sb")
                            nc.vector.tensor_copy(out=bc_sb, in_=bc_ps)
                            for ct in range(CT):
                                nc.vector.tensor_mul(
                                    out=ctx[ct][:, 1 + ls * 512: 1 + (ls + 1) * 512],
                                    in0=ctx_ps[ct], in1=bc_sb)

                    # ---- out conv + stats ----
                    o_sb = [op_pool.tile([128, L], F32, tag=f"o_{br}_{s}_{ct}", name=f"o_{br}_{s}_{ct}") for ct in range(CT)]
                    for ct in range(CT):
                        o_tiles[(br, s, ct)] = o_sb[ct]

                    def _acc(ct, ls, _br=br, _s=s):
                        i = ct * S * LS + _s * LS + ls
                        return slots[_br][:, i:i + 1]

                    conv_cc(o_sb, w_cc[f"wo_{abr}"], bias[f"ob_{abr}"], ctx, stats=_acc)
                    for ct in range(CT):
                        for ls in range(LS):
                            sq = sm.tile([128, 512], F32, tag="sqscr", name="sq")
                            osl = o_sb[ct][:, ls * 512:(ls + 1) * 512]
                            nc.vector.tensor_mul(out=sq, in0=osl, in1=osl)
                            i = (2 + ct) * S * LS + s * LS + ls
                            nc.vector.reduce_sum(
                                out=slots[br][:, i:i + 1], in_=sq,
                                axis=mybir.AxisListType.X)

                # ---- cross-core stats all-reduce for this branch ----
                statp = st.tile([128, 4], F32, tag=f"statp{br}")
                nc.vector.reduce_sum(out=statp, in_=slots[br].rearrange("p (g i) -> p g i", i=S * LS), axis=mybir.AxisListType.X)
                cc_in = dram.tile([128, 4], F32, tag=f"ccin{br}")
                cc_out = dram.tile([128, 4], F32, tag=f"ccout{br}")
                nc.sync.dma_start(out=cc_in, in_=statp)
                if os.environ.get("KERNEL_NO_CC"):
                    nc.sync.dma_start(out=cc_out, in_=cc_in)
                else:
                    nc.gpsimd.collective_compute(
                        "AllReduce", OP.add,
                        replica_groups=[list(range(NCORES))],
                        ins=[cc_in.opt()], outs=[cc_out.opt()])
                if br == 0:
                    # readback issued HERE (before branch b's collective in
                    # program order) so its semaphore wait covers only this
                    # branch's AllReduce.
                    sg = st.tile([128, 4], F32, tag=f"statg{br}")
                    nc.sync.dma_start(out=sg, in_=cc_out)
                    statg[br] = sg
                else:
                    statg[br] = cc_out

            # ---- BN finalize + residual + output ----
            # Branch a's finalize is issued BEFORE branch b's statg readback
            # so it executes during branch b's AllReduce latency.
            def finalize(br):
                abr = "a" if br == 0 else "b"
                if br == 0:
                    sg = statg[br]
                else:
                    sg = st.tile([128, 4], F32, tag=f"statg{br}")
                    nc.sync.dma_start(out=sg, in_=statg[br])
                mean = st.tile([128, CT], F32, tag=f"mean{br}")
                nc.vector.tensor_scalar_mul(mean, sg[:, 0:2], 1.0 / NSTAT)
                esq = st.tile([128, CT], F32, tag=f"esq{br}")
                nc.vector.tensor_scalar_mul(esq, sg[:, 2:4], 1.0 / NSTAT)
                m2 = st.tile([128, CT], F32, tag=f"m2{br}")
                nc.vector.tensor_mul(out=m2, in0=mean, in1=mean)
                var = st.tile([128, CT], F32, tag=f"var{br}")
                nc.vector.tensor_sub(out=var, in0=esq, in1=m2)
                sd = st.tile([128, CT], F32, tag=f"sd{br}")
                nc.scalar.activation(out=sd, in_=var, func=AF.Sqrt, bias=eps_sb[:, 0:1], scale=1.0)
                rstd = st.tile([128, CT], F32, tag=f"rstd{br}")
                nc.vector.reciprocal(out=rstd, in_=sd)
                A_ = st.tile([128, CT], F32, tag=f"A{br}")
                nc.vector.tensor_mul(out=A_, in0=rstd, in1=gb[f"ga_{abr}"])
                mA = st.tile([128, CT], F32, tag=f"mA{br}")
                nc.vector.tensor_mul(out=mA, in0=mean, in1=A_)
                Bc = st.tile([128, CT], F32, tag=f"Bc{br}")
                nc.vector.tensor_sub(out=Bc, in0=gb[f"be_{abr}"], in1=mA)
                for s in range(S):
                    for ct in range(CT):
                        tmp = bn.tile([128, L], F32, tag="bnt", name="bnt")
                        nc.scalar.activation(
                            out=tmp, in_=o_tiles[(br, s, ct)],
                            func=AF.Identity,
                            scale=A_[:, ct:ct + 1], bias=Bc[:, ct:ct + 1])
                        res = bn.tile([128, L], F32, tag="bnr", name="bnr")
                        nc.vector.tensor_add(
                            out=res, in0=tmp, in1=img[s][ct][:, 1:L + 1])
                        nc.sync.dma_start(out=out_p[s, br, ct], in_=res)

            finalize(0)
            finalize(1)

    nc.compile()
    return nc


def _get_nc():
    if not _NC_CACHE:
        _NC_CACHE.append(_build_nc())
    return _NC_CACHE[0]


def _prep_shared(inp):
    f = NP_MM
    m = {}

    def cc_layout(w):
        # [256(mt*128+mi), 256(kt*128+ki), 3(t)] -> [kt, ki, 768=(t*2+mt)*128+mi]
        return np.ascontiguousarray(
            np.asarray(w).reshape(2, 128, 2, 128, 3).transpose(2, 3, 4, 0, 1).reshape(2, 128, 768)).astype(f)

    m["wq_a"] = cc_layout(inp["a_qw"])
    m["wo_a"] = cc_layout(inp["a_ow"])
    m["wk_b"] = cc_layout(inp["b_kw"])
    m["wo_b"] = cc_layout(inp["b_ow"])
    # b_vw [c, c', t] -> [kt, t, ki, c]
    m["wv_b"] = np.ascontiguousarray(
        np.asarray(inp["b_vw"]).reshape(C, 2, 128, 3).transpose(1, 3, 2, 0)).astype(f)
    # rank-4 augmented weights: [C, 4] = [3 taps, bias], sliced by kt
    wk4 = np.concatenate(
        [np.asarray(inp["a_kw"])[:, 0, :], np.asarray(inp["a_kb"])[:, None]], axis=1)
    m["wk4_a"] = np.ascontiguousarray(wk4.reshape(CT, 128, 4)).astype(f)
    wq4 = np.concatenate(
        [np.asarray(inp["b_qw"])[:, 0, :], np.asarray(inp["b_qb"])[:, None]], axis=1)
    m["wq4_b"] = np.ascontiguousarray(wq4.reshape(CT, 128, 4)).astype(f)
    # [4, C] = [vb; 3 v-taps] -- row 0 pairs with the ones column of clit4
    # (PSUM partition slices must start at partition 0, so the denominator
    # row of H must be row 0)
    wv4 = np.concatenate(
        [np.asarray(inp["a_vb"])[None, :], np.asarray(inp["a_vw"])[:, 0, :].T], axis=0)
    m["wv4_a"] = np.ascontiguousarray(wv4).astype(f)
    for dst, src in (("qb_a", "a_qb"), ("ob_a", "a_ob"),
                     ("kb_b", "b_kb"), ("ob_b", "b_ob"),
                     ("ga_a", "a_g"), ("be_a", "a_beta"),
                     ("ga_b", "b_g"), ("be_b", "b_beta")):
        m[dst] = np.ascontiguousarray(np.asarray(inp[src]).reshape(2, 128).T).astype(np.float32)
    m["vb_b"] = np.ascontiguousarray(np.asarray(inp["b_vb"])[None, :]).astype(np.float32)
    return m


def _core_maps(image, clinical, shared, ncores=NCORES):
    in_maps = []
    for core in range(ncores):
        m = dict(shared)
        sl = slice(core * S, (core + 1) * S)
        a = image[sl].reshape(S, CT, 128, L)
        pad = np.zeros((S, CT, 128, L + 2), np.float32)
        pad[..., 1:L + 1] = a
        m["img"] = pad
        m["imm"] = pad.astype(NP_MM)
        c = clinical[sl][:, 0, :]
        im2 = np.zeros((S, 4, L), np.float32)
        im2[:, 0, 1:] = c[:, :L - 1]
        im2[:, 1, :] = c
        im2[:, 2, :L - 1] = c[:, 1:]
        im2[:, 3, :] = 1.0
        m["cli4"] = im2.astype(NP_MM)
        # transposed: [S, 128, 32] = per-m-position 4 values [1, taps...],
        # 8 mt-groups side by side (column 0 of each group = ones ->
        # denominator row of H)
        im2t = np.concatenate([im2[:, 3:4, :], im2[:, 0:3, :]], axis=1)
        m["clit4"] = np.ascontiguousarray(
            im2t.transpose(0, 2, 1).reshape(S, 8, 128, 4)
            .transpose(0, 2, 1, 3).reshape(S, 128, 32)).astype(NP_MM)
        in_maps.append(m)
    return in_maps


def kernel(**inputs):
    inp = {k: np.asarray(v) for k, v in inputs.items()}
    nc = _get_nc()
    shared = _prep_shared(inp)
    image = inp["image"].astype(np.float32)
    clinical = inp["clinical"].astype(np.float32)
    in_maps = _core_maps(image, clinical, shared)
    res = run_bass_kernel_spmd(nc, in_maps, core_ids=list(range(NCORES)))
    outs = np.concatenate([res.results[i]["out"] for i in range(NCORES)], axis=0)
    return np.ascontiguousarray(outs.reshape(16, 512, L))
